# revision 1
# baseline (speedup 1.0000x reference)
"""Trainium2 Bass kernel for nn_Co_Pam_Module (PAM-style sparse attention +
nearest-upsample + BatchNorm residual).

Sharding: data-parallel over batch B=8 across 8 NeuronCores (one batch per
core); BN batch statistics are synchronized with a tiny AllReduce.

Math (validated vs reference, rel err ~1e-6 in numpy):
  q = wq@y + bq            [32, 2048]
  k = wk@y + bk            [32, 2048]
  E^T[t,s] = sum_d k[d,t] q[d,s]        (energy transposed; range ~±31 so
  P^T = exp(E^T)                         no max-subtraction is needed in f32)
  x_pool[c,j] = sum_u x[c,4j+u]
  vmm = (gamma*wv) @ x_pool             (gamma folded into weights)
  O~g[c,i] = sum_t vmm^T[t,c]*P^T[t,i]  via matmul with vpT=[vmm^T | ones];
  s[i]    = column 64 of the same accumulation (softmax denominator)
  G = O~g/s + 4*gamma*bv ; sync-BN stats via AllReduce of (sum G, sum G^2)
  out = x + scale_c*G_rep4 + bias_c
"""

import numpy as np

import concourse.bass as bass
import concourse.tile as tile
from concourse import mybir
from concourse.vector_clock import ScopedClock

F32 = mybir.dt.float32
F32R = mybir.dt.float32r
AF = mybir.ActivationFunctionType
ALU = mybir.AluOpType

SYNC_MODE = "collective"  # "collective" | "rdma" | "none"

B, CX, HX, WX = 8, 64, 128, 64
CY, HY, WY = 256, 64, 32
SX, SY, D, RUP = HX * WX, HY * WY, 32, 4  # 8192, 2048, 32, 4
N_CORES = 8
BN_EPS = 1e-5
WPK_COLS = 840


# ---------------------------------------------------------------------------
# Workaround: walrus in this container rejects >cap sem waits on the Tile
# kernel-tail Drain.  Emit explicit per-sem wait_ge instructions instead.
def _patched_drain_and_barrier(self, tick_clock, wait_clock):
    nc = self.nc
    probe = nc.sync.nop(nofuse=True)
    wait_clock.add_sem_waits(probe.ins, ScopedClock({None: tick_clock.global_clock}))
    waits = list(probe.ins.sync_info.on_wait)
    probe.ins.sync_info.on_wait = []
    name2handle = {}
    for k, h in wait_clock.sems.allocated().items():
        name2handle[getattr(h, "name", str(k))] = h
    for w in waits:
        h = name2handle.get(w.ant_name)
        if h is None:
            raise RuntimeError(f"no sem handle for {w.ant_name}")
        nc.sync.wait_ge(h, w.wait_value)
    nc.sync.drain()
    nc.all_engine_barrier()
    popped = nc._tile_sem_poison_stack.pop()
    assert popped is self._sem_poison
    nc.clear_and_free_semaphores(list(self.sems.allocated().values()))
    nc.all_engine_barrier()


tile.TileContext._drain_and_barrier = _patched_drain_and_barrier


def _split_excess_waits(nc, cap=1):
    """Walrus in this container allows only `cap` sem waits per instruction.
    Hoist excess semaphore waits onto same-engine NoOps inserted just before
    the instruction (same engine + program order => semantics preserved)."""
    n_split = 0
    for f in nc.m.functions:
        for blk in f.blocks:
            insts = list(blk.instructions)
            new_insts = []
            changed = False
            for inst in insts:
                si = inst.sync_info
                waits = list(si.on_wait) if si is not None else []
                if len(waits) > cap:
                    sem_w = [w for w in waits if w.sync_type == "semaphore"]
                    other_w = [w for w in waits if w.sync_type != "semaphore"]
                    budget = max(0, cap - len(other_w))
                    keep, excess = sem_w[:budget], sem_w[budget:]
                    for i in range(0, len(excess), max(1, cap)):
                        chunk = excess[i : i + max(1, cap)]
                        nop = mybir.InstNoOp(
                            name=f"{inst.name}-ws{n_split}",
                            sync_info=mybir.SyncInfo(on_wait=chunk, on_update=[]),
                            bass_nofuse=True,
                            engine=inst.engine,
                        )
                        new_insts.append(nop)
                        n_split += 1
                    si.on_wait = other_w + keep
                    changed = True
                new_insts.append(inst)
            if changed:
                blk.instructions = new_insts
    return n_split
# ---------------------------------------------------------------------------


def _rep_ap(ap, rep):
    """Append a step-0 (repeat) innermost free dim to an AP."""
    return bass.AP(tensor=ap.tensor, offset=ap.offset, ap=list(ap.ap) + [[0, rep]])


def build_module(split_waits=True):
    nc = bass.Bass()

    xb = nc.dram_tensor("xb", [128, SX // 2], F32, kind="ExternalInput")
    yb = nc.dram_tensor("yb", [2, 128, SY], F32R, kind="ExternalInput")
    # packed small weights, one DMA: see _host_inputs for the column map
    wpk = nc.dram_tensor("wpk", [128, WPK_COLS], F32R, kind="ExternalInput")
    msc = nc.dram_tensor("msc", [128, 8], F32, kind="ExternalInput")
    out = nc.dram_tensor("out", [128, SX // 2], F32, kind="ExternalOutput")

    with tile.TileContext(nc, num_cores=N_CORES) as tc:
        with (
            tc.tile_pool(name="const", bufs=1) as cp,
            tc.tile_pool(name="big", bufs=1) as big,
            tc.tile_pool(name="ptile", bufs=6) as pp,
            tc.tile_pool(name="dram", bufs=1, space="DRAM") as dp,
        ):
            # ---------------- constants / weights (single DMA) ----------------
            wpk_sb = cp.tile([128, WPK_COLS], F32R)
            nc.sync.dma_start(wpk_sb[:, 0:768], wpk[:, 0:768])
            # column map (f32 cols): 0:256 wqT(kc0,kc1), 256:512 wkT,
            # 512:640 bq row, 640:768 bk row, 768:832 wvT*gamma (stacked
            # twice on partitions); first DMA piece carries everything the
            # q/k matmuls need
            bq_sb = wpk_sb[0:1, 512:640]
            bk_sb = wpk_sb[0:1, 640:768]
            wv_sb = wpk_sb[:, 768:832]
            msc_sb = cp.tile([128, 8], F32)
            bv4g_sb = msc_sb[0:64, 0:1]
            bv4g_sb2 = msc_sb[:, 0:1]  # [128,1]
            c_s1_sb = msc_sb[0:64, 1:2]
            bv4g2_sb = msc_sb[0:64, 2:3]
            c_s2_sb = msc_sb[0:64, 3:4]
            bnw_sb = msc_sb[:, 4:5]  # [128,1] both halves
            bnb_sb = msc_sb[:, 5:6]  # [128,1] both halves

            ones_row = cp.tile([1, 512], F32R)
            nc.vector.memset(ones_row[:].bitcast(F32), 1.0)
            ones64 = cp.tile([1, 64], F32R)
            nc.vector.memset(ones64[:].bitcast(F32), 1.0)
            eps_sb = cp.tile([128, 1], F32)
            nc.vector.memset(eps_sb[:], BN_EPS)

            # prewarm exp table early (overlaps initial DMA)
            warm = cp.tile([1, 8], F32)
            nc.vector.memset(warm[:], 0.0)
            nc.scalar.activation(warm[:], warm[:], AF.Exp)
            # prewarm the PE clock (HAM ramp): dummy matmul chain on a zero tile
            pewarm = cp.tile([128, 512], F32R)
            nc.vector.memset(pewarm[:].bitcast(F32), 0.0)

            # ---------------- big inputs ----------------
            y_sb = big.tile([128, 2, SY], F32R)
            # x in split layout: partition h*64+c holds x[c, 4096h:4096(h+1)]
            x2 = big.tile([128, SX // 2], F32)
            NXP = 4  # x pieces; pooling/vpT chunked to chase the DMA
            def y_quarter(ch):
                for kc in range(2):
                    nc.sync.dma_start(
                        y_sb[:, kc, ch * 512 : (ch + 1) * 512],
                        yb[kc][:, ch * 512 : (ch + 1) * 512],
                    )

            def x_piece(p):
                xsl = slice(p * 1024, (p + 1) * 1024)
                nc.sync.dma_start(x2[:, xsl], xb[:, xsl])

            # ordered so each consumer's operand lands just before its first
            # use: x pieces chase the vpT chain (iter 2p), late y quarters
            # only gate E at iters 8/12
            y_quarter(0)
            y_quarter(1)
            nc.sync.dma_start(wpk_sb[:, 768:], wpk[:, 768:])
            x_piece(0)
            x_piece(1)
            y_quarter(2)
            x_piece(2)
            x_piece(3)
            y_quarter(3)
            nc.sync.dma_start(msc_sb[:], msc[:])

            q_sb = big.tile([128, SY], F32R)
            k_sb = big.tile([128, SY], F32R)

            # ---------------- main compute: single PSUM regime ----------------
            # psE: 3 rotating [128,1024] slots (6 banks) shared by warmup/qk/
            # vpT/rs-broadcast/E tiles; psO: [65,1024] accumulator (2 banks).
            t1 = big.tile([128, SX // 4], F32)
            xp = big.tile([128, SX // 8], F32R)
            xv = x2[:].rearrange("p (n u) -> p n u", u=2)
            tv = t1[:].rearrange("p (n u) -> p n u", u=2)
            vpT = big.tile([128, 16, 65], F32R)
            nc.vector.memset(vpT[:, :, 64:65].bitcast(F32), 1.0)
            G0 = big.tile([64, SY], F32)
            s1_h = cp.tile([64, 2], F32)
            s2_h = cp.tile([64, 2], F32)
            junk2 = big.tile([64, 1024], F32, tag="junk2")
            with (
                tc.tile_pool(name="psE", bufs=3, space="PSUM") as psE,
                tc.tile_pool(name="psO", bufs=1, space="PSUM") as psO,
            ):
                # PE clock warmup
                wslot = psE.tile([128, 1024], F32, tag="E")
                for _ in range(4):
                    nc.tensor.matmul(
                        wslot[:, 0:512], pewarm[:, 0:128], pewarm[:],
                        start=True, stop=True,
                    )

                def emit_qk(w_off, b_t, dst, qt):
                    gslc = slice(qt * 512, (qt + 1) * 512)
                    ps = psE.tile([128, 1024], F32, tag="E")
                    for kc in range(2):
                        nc.tensor.matmul(
                            ps[:, 0:512],
                            wpk_sb[:, w_off + kc * 128 : w_off + kc * 128 + 128],
                            y_sb[:, kc, gslc],
                            start=(kc == 0),
                            stop=False,
                        )
                    nc.tensor.matmul(
                        ps[:, 0:512], b_t[:], ones_row[:],
                        start=False, stop=True,
                    )
                    nc.vector.tensor_copy(dst[:, gslc], ps[:, 0:512])

                emit_qk(0, bq_sb, q_sb, 0)
                emit_qk(256, bk_sb, k_sb, 0)
                emit_qk(0, bq_sb, q_sb, 1)

                def emit_pool_sub(jc):
                    # one 128-col xp window -> vpT chunks {2jc, 2jc+1}; spreads
                    # the piece work over two iterations to balance PE vs ACT
                    t1s = slice(jc * 256, (jc + 1) * 256)
                    nc.vector.tensor_add(t1[:, t1s], xv[:, t1s, 0], xv[:, t1s, 1])
                    xps = slice(jc * 128, (jc + 1) * 128)
                    nc.vector.tensor_add(xp[:, xps], tv[:, xps, 0], tv[:, xps, 1])
                    vps = psE.tile([128, 1024], F32, tag="E")
                    for hh in range(2):
                        base = slice(hh * 64, hh * 64 + 64)
                        # separate banks (cols 0 / 512): start=True bank clears
                        # cannot collide
                        nc.tensor.matmul(
                            vps[0:128, hh * 512 : hh * 512 + 64],
                            xp[base, xps], wv_sb[base, :],
                            start=True, stop=True,
                        )
                    vv = vps[:].rearrange("p (g c) -> p g c", c=512)
                    nc.vector.tensor_copy(vpT[:, 2 * jc : 2 * jc + 2, 0:64], vv[:, :, 0:64])

                def emit_half_tail(h, o_ps):
                    isl_g = slice(h * 1024, (h + 1) * 1024)
                    rs_sb = big.tile([1, 1024], F32R, tag="rs")
                    with nc.allow_low_precision(reason="fp32r softmax denominators"):
                        nc.vector.reciprocal(rs_sb[:], o_ps[64:65, :])
                    rr_ps = psE.tile([128, 1024], F32, tag="E")
                    for c2 in range(2):
                        nc.tensor.matmul(
                            rr_ps[0:64, c2 * 512 : (c2 + 1) * 512],
                            ones64[:],
                            rs_sb[:, c2 * 512 : (c2 + 1) * 512],
                            start=True, stop=True,
                        )
                    rr_sb = big.tile([64, 1024], F32, tag="rsrep_sb")
                    nc.vector.tensor_copy(rr_sb[:], rr_ps[0:64, :])
                    nc.vector.tensor_mul(G0[:, isl_g], o_ps[0:64, :], rr_sb[:])
                    nc.vector.tensor_reduce(
                        s1_h[:, h : h + 1], G0[:, isl_g],
                        mybir.AxisListType.X, ALU.add,
                    )

                def emit_half_sq(h):
                    isl_g = slice(h * 1024, (h + 1) * 1024)
                    nc.scalar.activation(
                        junk2[:], G0[:, isl_g], AF.Square,
                        accum_out=s2_h[:, h : h + 1],
                    )

                o_ps_prev = None
                for h in range(2):
                    o_ps = psO.tile([65, 1024], F32, tag="O")
                    for ts in range(16):
                        tslc = slice(ts * 128, (ts + 1) * 128)
                        e_ps = psE.tile([128, 1024], F32, tag="E")
                        for c2 in range(2):
                            nc.tensor.matmul(
                                e_ps[:, c2 * 512 : (c2 + 1) * 512],
                                k_sb[0:32, tslc],
                                q_sb[0:32, h * 1024 + c2 * 512 : h * 1024 + c2 * 512 + 512],
                                start=True, stop=True,
                            )
                        p_sb = pp.tile([128, 1024], F32R, tag="P")
                        nc.scalar.activation(p_sb[:], e_ps[:], AF.Exp)
                        if h == 0:
                            if ts < 8:
                                emit_pool_sub(ts)
                            if ts == 1:
                                emit_qk(256, bk_sb, k_sb, 1)
                            elif ts == 5:
                                emit_qk(256, bk_sb, k_sb, 2)
                            elif ts == 9:
                                emit_qk(256, bk_sb, k_sb, 3)
                            elif ts == 10:
                                emit_qk(0, bq_sb, q_sb, 2)
                            elif ts == 12:
                                emit_qk(0, bq_sb, q_sb, 3)

                        for c2 in range(2):
                            nc.tensor.matmul(
                                o_ps[:, c2 * 512 : (c2 + 1) * 512],
                                vpT[:, 2 * (ts % 8) + ts // 8, :],
                                p_sb[:, c2 * 512 : (c2 + 1) * 512],
                                start=(ts == 0),
                                stop=(ts == 15),
                                skip_group_check=True,
                            )
                        if h == 1 and ts == 1 and o_ps_prev is not None:
                            emit_half_tail(0, o_ps_prev)
                    o_ps_prev = o_ps
                emit_half_tail(1, o_ps_prev)
                emit_half_sq(0)
                emit_half_sq(1)

            # ---------------- BN stats + AllReduce ----------------
            s1_0 = cp.tile([64, 1], F32)
            s2_0 = cp.tile([64, 1], F32)
            nc.vector.tensor_add(s1_0[:], s1_h[:, 0:1], s1_h[:, 1:2])
            nc.vector.tensor_add(s2_0[:], s2_h[:, 0:1], s2_h[:, 1:2])
            ar_sb = cp.tile([64, 2], F32)
            # s1 = s1_0 + 2048*bv4g
            nc.vector.tensor_add(ar_sb[:, 0:1], s1_0[:], c_s1_sb[:])
            # s2 = s2_0 + 2*bv4g*s1_0 + 2048*bv4g^2
            t2 = cp.tile([64, 1], F32)
            nc.vector.tensor_scalar(
                t2[:], s1_0[:], bv4g2_sb[:], c_s2_sb[:], ALU.mult, ALU.add
            )
            nc.vector.tensor_add(ar_sb[:, 1:2], s2_0[:], t2[:])

            # pre-move G0-hi to partitions 64:128 (hidden under the AllGather)
            G2hi = big.tile([128, 1024], F32, tag="g2hi")
            nc.sync.dma_start(G2hi[64:128, :], G0[:, 1024:2048])

            if SYNC_MODE == "collective":
                sums_tile = cp.tile([128, 2], F32)
                ar_in = dp.tile([64, 2], F32)
                ar_out = dp.tile([N_CORES, 64, 2], F32)
                nc.sync.dma_start(ar_in[:], ar_sb[:])
                nc.gpsimd.collective_compute(
                    "AllGather",
                    ALU.bypass,
                    ins=[ar_in.opt()],
                    outs=[ar_out.opt()],
                    replica_groups=[list(range(N_CORES))],
                )
                # load gathered as [128(dup), 2, 8] (both halves) + rank-reduce
                gath_sb = cp.tile([128, 2, N_CORES], F32)
                for hh in range(2):
                    nc.sync.dma_start(
                        gath_sb[hh * 64 : hh * 64 + 64, :, :],
                        ar_out[:].rearrange("r c j -> c j r"),
                    )
                nc.vector.tensor_reduce(
                    sums_tile[:], gath_sb[:], mybir.AxisListType.X, ALU.add
                )
                sums_sb = sums_tile[:]
            elif SYNC_MODE == "rdma":
                # recursive-doubling allreduce over same-chip peers: 3 rounds of
                # XOR-relative remote DMA broadcasts (single real dest each).
                acc = cp.tile([128, 2], F32)
                nc.vector.memset(acc[:], 0.0)
                nc.vector.tensor_copy(acc[0:64, :], ar_sb[:])
                recvs = cp.tile([128, 3, 2], F32)
                rsems = [nc.alloc_semaphore(f"rdma_r{r}") for r in range(3)]
                lsem = nc.alloc_semaphore("rdma_l")
                with tc.tile_critical():
                    g = nc.gpsimd
                    for r in range(3):
                        delta = 1 << r
                        slot = 4 if (delta & 4) else 0
                        rd = [None] * 8
                        rd[slot] = (0, delta)
                        g.remote_dma_broadcast(
                            recvs[:, r, :],
                            acc[:],
                            remote_sem=rsems[r],
                            local_sem=lsem,
                            rdests=rd,
                        )
                        g.trigger_dma(count=None)
                        g.wait_ge(lsem, 16 * (r + 1))
                        g.wait_ge(rsems[r], 2)
                        g.tensor_add(acc[:], acc[:], recvs[:, r, :])
                sums_sb = acc[0:64, :]
            else:
                # debug fallback: per-core stats scaled by B (exact only if all
                # batches had identical stats)
                sums_tile = cp.tile([128, 2], F32)
                bounce = dp.tile([64, 2], F32)
                nc.sync.dma_start(bounce[:], ar_sb[:])
                for hh in range(2):
                    nc.sync.dma_start(
                        sums_tile[hh * 64 : hh * 64 + 64, :], bounce[:]
                    )
                nc.vector.tensor_scalar_mul(sums_tile[:], sums_tile[:], float(B))
                sums_sb = sums_tile[:]

            # ---------------- scale/bias + final combine ----------------
            sq_warm = cp.tile([64, 1], F32)
            nc.scalar.activation(sq_warm[:], s2_0[:], AF.Sqrt, bias=eps_sb[0:64, :])
            mm_sb = cp.tile([128, 2], F32)
            nc.vector.tensor_scalar_mul(mm_sb[:], sums_sb, float(RUP) / (B * SX))
            m_ap = mm_sb[:, 0:1]
            msq_ap = mm_sb[:, 1:2]
            var_sb = cp.tile([128, 1], F32)
            m2_sb = cp.tile([128, 1], F32)
            nc.vector.tensor_mul(m2_sb[:], m_ap, m_ap)
            nc.vector.tensor_sub(var_sb[:], msq_ap, m2_sb[:])
            std_sb = cp.tile([128, 1], F32)
            nc.scalar.activation(std_sb[:], var_sb[:], AF.Sqrt, bias=eps_sb[:])
            rstd_sb = cp.tile([128, 1], F32)
            nc.vector.reciprocal(rstd_sb[:], std_sb[:])
            scale_sb = cp.tile([128, 1], F32)
            nc.vector.tensor_mul(scale_sb[:], rstd_sb[:], bnw_sb[:])
            # bias2 = bnb - m*scale + scale*bv4g   (both partition halves)
            tb = cp.tile([128, 1], F32)
            nc.vector.tensor_mul(tb[:], m_ap, scale_sb[:])
            bias2_sb = cp.tile([128, 1], F32)
            nc.vector.tensor_sub(bias2_sb[:], bnb_sb[:], tb[:])
            tb2 = cp.tile([128, 1], F32)
            nc.vector.tensor_mul(tb2[:], scale_sb[:], bv4g_sb2)
            nc.vector.tensor_add(bias2_sb[:], bias2_sb[:], tb2[:])

            # R2 split layout [128, 1024]: R2[h*64+c, i'] = scale*G0[c, 1024h+i'] + bias2
            # G0-hi is pre-moved to partitions 64:128 during the AllGather (hidden)
            R2 = big.tile([128, 1024], F32)
            nc.vector.tensor_scalar(
                R2[64:128, :], G2hi[64:128, :], scale_sb[64:128, :],
                bias2_sb[64:128, :], ALU.mult, ALU.add,
            )
            nc.vector.tensor_scalar(
                R2[0:64, :], G0[:, 0:1024], scale_sb[0:64, :],
                bias2_sb[0:64, :], ALU.mult, ALU.add,
            )

            # out2[p, f] = x2[p, f] + R2[p, f>>2]
            out2 = big.tile([128, SX // 2], F32)
            o_view = out2[:].rearrange("p (n u) -> p n u", u=4)
            x_view = x2[:].rearrange("p (n u) -> p n u", u=4)
            NFC = 8
            csz = (SX // 2) // NFC  # 512 output cols -> 128 R cols per chunk
            for qc in range(NFC):
                nsl = slice(qc * (csz // 4), (qc + 1) * (csz // 4))
                nc.vector.tensor_add(
                    o_view[:, nsl, :],
                    x_view[:, nsl, :],
                    _rep_ap(R2[:, nsl], 4),
                )
                nc.sync.dma_start(
                    out[:, qc * csz : (qc + 1) * csz],
                    out2[:, qc * csz : (qc + 1) * csz],
                )

    if split_waits:
        _split_excess_waits(nc)
    return nc


def _host_inputs(x, y, wq, bq, wk, bk, wv, bv, gamma, bn_w, bn_b):
    g = float(np.asarray(gamma).reshape(-1)[0])
    wqT_rep = np.tile(np.ascontiguousarray(wq.T), (1, 4))  # [256, 128]
    wkT_rep = np.tile(np.ascontiguousarray(wk.T), (1, 4))
    bv4g = (4.0 * g * bv)
    wpk = np.zeros((128, WPK_COLS), np.float32)
    wpk[:, 0:128] = wqT_rep[0:128]
    wpk[:, 128:256] = wqT_rep[128:256]
    wpk[:, 256:384] = wkT_rep[0:128]
    wpk[:, 384:512] = wkT_rep[128:256]
    wpk[0, 512:640] = np.tile(bq, 4)
    wpk[0, 640:768] = np.tile(bk, 4)
    wpk[0:64, 768:832] = (g * wv).T
    wpk[64:128, 768:832] = (g * wv).T
    msc = np.zeros((128, 8), np.float32)
    for hh in range(2):
        msc[hh * 64 : hh * 64 + 64, 0] = bv4g
        msc[hh * 64 : hh * 64 + 64, 1] = SY * bv4g
        msc[hh * 64 : hh * 64 + 64, 2] = 2.0 * bv4g
        msc[hh * 64 : hh * 64 + 64, 3] = SY * bv4g * bv4g
        msc[hh * 64 : hh * 64 + 64, 4] = bn_w
        msc[hh * 64 : hh * 64 + 64, 5] = bn_b
    common = {"wpk": wpk, "msc": msc}
    in_maps = []
    for b in range(B):
        m = dict(common)
        # split layout: [2, 64, 4096] where [h, c, f] = x[b, c, 4096h + f]
        xf = np.asarray(x[b], np.float32).reshape(64, 2, SX // 2).transpose(1, 0, 2)
        m["xb"] = np.ascontiguousarray(xf.reshape(128, SX // 2))
        m["yb"] = np.ascontiguousarray(
            np.asarray(y[b], np.float32).reshape(2, 128, SY)
        )
        in_maps.append(m)
    return in_maps


_NC_CACHE = {}


def kernel(x, y, wq, bq, wk, bk, wv, bv, gamma, bn_w, bn_b, _trace=False):
    from concourse.bass_utils import run_bass_kernel_spmd

    if "nc" not in _NC_CACHE:
        _NC_CACHE["nc"] = build_module()
    nc = _NC_CACHE["nc"]
    in_maps = _host_inputs(x, y, wq, bq, wk, bk, wv, bv, gamma, bn_w, bn_b)
    res = run_bass_kernel_spmd(
        nc, in_maps, core_ids=list(range(N_CORES)), trace=_trace
    )
    out = np.empty((B, CX, HX, WX), np.float32)
    for b in range(B):
        o2 = res.results[b]["out"].reshape(2, CX, SX // 2)
        out[b] = o2.transpose(1, 0, 2).reshape(CX, HX, WX)
    if _trace:
        _NC_CACHE["last_results"] = res
    return out

